# revision 2
# baseline (speedup 1.0000x reference)
"""Averaged Hausdorff loss kernel for 8 Trainium2 NeuronCores — v2.

Same exact windowed nearest-neighbor formulation as v1 (KD leaves of 128
rows, host-certified window, host fixup of uncertified rows), with the
device program restructured around the measured bottlenecks:

- W=160 candidates per leaf (was 384): one [13,128]x[13,160] matmul per
  leaf, 16 total per core, alternating PE row groups 0/32.  ~12% of rows
  fail certification at W=160 on N(0,1)^3 data and are recomputed on the
  host (exact either way; certification makes this exact for any input).
- PSUM tile per 2-pair group = 2 banks: bank A holds the row-group-0
  chunks of both pairs (offsets 0/256), bank B the row-group-1 chunks,
  so the two concurrent matmul streams never share a bank (sharing one
  faults) and the 4 chunks sit at uniform stride 256 for rank-3 APs.
  ScalarE stages the second 80 of each 160-chunk to SBUF; the fused
  dual-port DVE min-scan (MINSCAN_SEG_ANT) reduces 80 PSUM + 80 SBUF
  element pairs per leaf at 2 fp32/cycle.
- Inputs ride all 3 DMA queues: sync/scalar (HWDGE) carry direction 1
  split early (pairs 0-1) + rest, gpsimd (SWDGE) carries direction 2 in
  parallel.  Within a group both bank-A matmuls are emitted before the
  bank-B ones (plane-1 data lands ~0.9us after plane 0).  Outputs are 4
  small DMAs alternating sync/scalar so the last waits only on the
  final DVE op.  PE warm-up was tried and removed: this part pins the
  PE at ~1.2 GHz regardless of HAM activity.
"""

import sys

sys.path.insert(0, "/opt/trn_rl_repo")

import numpy as np

N_CORES = 8
N = 8192          # set1 rows
M = 8192          # set2 rows
D = 3
ROWS_PER_CORE = N // N_CORES          # 1024
BLOCKS = ROWS_PER_CORE // 128         # 8 KD leaves per core per direction
NLEAF = N // 128                      # 64 leaves total per direction
W = 160                               # candidate window per leaf
K = 13                                # augmented contraction dim
FP32_MAX = 3.4e38
PAIR_COLS = 128 + W                   # lhs + rhs columns per leaf
DIR_COLS = (BLOCKS // 2) * PAIR_COLS  # 4 pairs per plane per direction
TOT = 2 * DIR_COLS                    # plane width
EARLY = 2 * PAIR_COLS                 # first two pairs of direction 1
N_WARM = 12                           # PE warm-up matmuls

_compiled = None


def _register_minseg():
    """Fused segmented DVE op (identical to v1): per-segment running
    min-scan over min(in0[p,..,k], in1[p,..,k]), reseeded from s0 at every
    innermost-dim (SUB_DIM) boundary.  Written through an AP whose
    innermost dim is stride-0, so each segment's destination cell ends
    with that segment's total min."""
    from concourse import dve_ops
    from concourse import dve_spec as ds
    from concourse.dve_uop import DveOpSpec

    def _ref(in0, in1, c0, c1, c2):
        b = np.minimum(in0.astype(np.float32), in1.astype(np.float32))
        P = b.shape[0]
        flat = b.reshape(P, -1, b.shape[-1])
        init = np.full((P, flat.shape[1], 1), c0, np.float32)
        out = np.minimum.accumulate(
            np.concatenate([init, flat], axis=-1), axis=-1
        )[:, :, 1:]
        return out.reshape(b.shape)

    name = "MINSCAN_SEG_ANT"
    if name in dve_ops._SUB_OPCODE_FOR_NAME:
        return next(op for op in dve_ops.OPS if op.name == name)

    body = ds.scan(ds.AluOp.MIN, ds.minn(ds.Src0, ds.Src1), init=ds.C0)
    spec = ds.Spec(body=body, reference=_ref)

    def lower_seg(ver):
        n_lanes, n_stages = ds.N_LANES[ver], ds.N_STAGES[ver]
        ds._validate_body(spec, ver)
        sp = ds._hoist_stream_invariant_ops(spec)
        scans = ds._collect(sp.body, ds.Scan)
        latches = ds._collect(sp.body, ds.Latch)
        placement = ds._build_placement(sp, scans, n_stages, n_lanes)
        states = ds._build_state_machine(sp, scans, latches, placement)
        assert len(states) == 2, states  # [seed, steady]
        seed, steady = states
        d = placement.node_stage[scans[0]]
        steady2 = ds._State(
            placement=placement,
            consume=steady.consume,
            trigger=(
                ds.Trigger.SRC_TENSOR_DONE,
                ds.Trigger.SUB_DIM_DONE,
                ds.Trigger.NONE,
            ),
            next=(0, 2, 0),
        )
        step = ds._State(
            placement=placement,
            consume=steady.consume,
            overrides={d: ds._Stage(scans[0].op, ds.C0, scans[0].expr)},
            trigger=(
                ds.Trigger.SRC_TENSOR_DONE,
                ds.Trigger.SUB_DIM_DONE,
                ds.Trigger.COUNT,
            ),
            next=(0, 2, 1),
            repeat=1,
        )
        uops = [ds._assemble(s) for s in (seed, steady2, step)]
        for u in uops:
            u.validate(ver)
        return uops

    op = dve_ops.DveOp(name, spec, subdim=True, uops_sha={})
    dve_ops.OPS.append(op)
    dve_ops._SUB_OPCODE_FOR_NAME[name] = (
        dve_ops._CUSTOM_DVE_ROW_BASE + len(dve_ops.OPS) - 1
    )
    assert dve_ops._SUB_OPCODE_FOR_NAME[name] < 0x20
    dve_ops.CUSTOM_DVE_SPECS[name] = spec
    for ver in ("v3", "v4"):
        compiled = DveOpSpec(
            name=name,
            opcode=dve_ops.get_dve_sub_opcode(name),
            uops=lower_seg(ver),
            rd1_en=True,
        )
        op.uops_sha[ver] = compiled.sha(ver)
        dve_ops._COMPILE_CACHE[(name, ver)] = compiled
    return op


def _build_program():
    import concourse.tile as tile
    from concourse import bacc, mybir

    minseg = _register_minseg()

    nc = bacc.Bacc("TRN2", target_bir_lowering=False, debug=False)
    f32 = mybir.dt.float32
    f16 = mybir.dt.float16

    KR = 32 + K   # SBUF operand stack height (rows 0..12 and 32..44)
    H = W // 2    # half-chunk length for the dual-port scan

    in0_d = nc.dram_tensor("in0", [K, TOT], f16, kind="ExternalInput")
    in1_d = nc.dram_tensor("in1", [K, TOT], f16, kind="ExternalInput")
    out_d = nc.dram_tensor("out", [128, 16], f32, kind="ExternalOutput")

    with tile.TileContext(nc) as tc:
        with (
            tc.tile_pool(name="ops", bufs=1) as ops,
            tc.tile_pool(name="ps", bufs=3, space="PSUM") as ps,
            tc.tile_pool(name="sc", bufs=3) as scp,
            tc.tile_pool(name="small", bufs=1) as small,
        ):
            # --- input tiles: one per (plane, piece) ---
            # e  = dir-1 pairs 0-1, ra = dir-1 pairs 2-3 (HWDGE queues),
            # rb = all of dir-2 (gpsimd SWDGE, issued in parallel).
            t0e = ops.tile([KR, EARLY], f16, tag="t0e")
            t0ra = ops.tile([KR, DIR_COLS - EARLY], f16, tag="t0ra")
            t0rb = ops.tile([KR, DIR_COLS], f16, tag="t0rb")
            t1e = ops.tile([KR, EARLY], f16, tag="t1e")
            t1ra = ops.tile([KR, DIR_COLS - EARLY], f16, tag="t1ra")
            t1rb = ops.tile([KR, DIR_COLS], f16, tag="t1rb")

            nc.sync.dma_start(t0e[0:K, :], in0_d[:, 0:EARLY])
            nc.scalar.dma_start(t1e[32 : 32 + K, :], in1_d[:, 0:EARLY])
            nc.gpsimd.dma_start(t0rb[0:K, :], in0_d[:, DIR_COLS:TOT])
            nc.gpsimd.dma_start(t1rb[32 : 32 + K, :], in1_d[:, DIR_COLS:TOT])
            nc.sync.dma_start(t0ra[0:K, :], in0_d[:, EARLY:DIR_COLS])
            nc.scalar.dma_start(t1ra[32 : 32 + K, :], in1_d[:, EARLY:DIR_COLS])

            def cols(o, p):
                """(col offset within piece, piece index 0=e/1=ra/2=rb)
                for direction o, pair p."""
                if o == 1:
                    return p * PAIR_COLS, 2
                if p < 2:
                    return p * PAIR_COLS, 0
                return p * PAIR_COLS - EARLY, 1

            # --- main loop: 2 dirs x 2 groups of 2 pairs ---
            # PSUM tile [128, 4, 256] = 2 banks per group: slots 0/1 =
            # bank A (row-group-0 chunks of pairs a/b), slots 2/3 = bank
            # B (row-group-1 chunks).  The two concurrent matmul streams
            # always write different banks; sequential same-group
            # matmuls share a bank at offsets 0/256 (legal).  Slot s ->
            # local leaf PERM[s] = [0, 2, 1, 3][s] (host un-permutes).
            p0 = (t0e, t0ra, t0rb)
            p1 = (t1e, t1ra, t1rb)
            for o in range(2):
                for g in range(2):
                    pk = ps.tile([128, 4, 256], f32, name="pk", tag="pk")
                    sc = scp.tile([128, 4, H], f32, name="sc", tag="sc")
                    rm = small.tile(
                        [128, 4], f32, name=f"rm{o}{g}", tag=f"rm{o}{g}"
                    )
                    # bank-A chunks (plane 0) for both pairs first, then
                    # bank-B (plane 1): plane-1 data arrives ~0.9us later
                    # than plane 0, so this keeps the column stream busy
                    # on plane-0 work while plane 1 lands.
                    for c in range(2):
                        te = (p0, p1)[c]
                        rows = slice(32 * c, 32 * c + K)
                        for j in range(2):
                            p = 2 * g + j
                            off, piece = cols(o, p)
                            t = te[piece]
                            l0 = slice(off, off + 128)
                            r0 = slice(off + 128, off + 128 + W)
                            nc.tensor.matmul(
                                pk[:, 2 * c + j, 0:W], t[rows, l0], t[rows, r0]
                            )
                    # second halves of the 4 chunks -> SBUF (ScalarE)
                    nc.scalar.copy(sc[:], pk[:, :, H:W])
                    # fused dual-port segmented min-scan -> 4 row-min cells
                    nc.vector._custom_dve(
                        minseg,
                        out=rm[:].broadcast_to((128, 4, H)),
                        in0=pk[:, :, 0:H],
                        in1=sc[:],
                        s0=FP32_MAX,
                    )
                    oq = nc.sync if g == 0 else nc.scalar
                    oq.dma_start(
                        out_d[:, 8 * o + 4 * g : 8 * o + 4 * g + 4], rm[:]
                    )

    nc.compile()
    return nc


def _get_program():
    global _compiled
    if _compiled is None:
        _compiled = _build_program()
    return _compiled


def _split16(v):
    """fp64 vector -> (hi, lo) fp16 with v ~= hi + lo to ~2^-22 rel."""
    hi = v.astype(np.float16)
    lo = (v - hi.astype(np.float64)).astype(np.float16)
    return hi.astype(np.float64), lo.astype(np.float64)


def _aug_stacks(s64):
    """[n, 3] fp64 -> ([13, n] lhs stack, [13, n] rhs stack) fp16."""
    n = (s64 * s64).sum(axis=1)
    ones = np.ones(s64.shape[0], dtype=np.float64)
    xh = [None] * D
    xl = [None] * D
    for d in range(D):
        xh[d], xl[d] = _split16(s64[:, d])
    nh, nl = _split16(n)
    lhs = np.stack(
        [xh[0], xh[1], xh[2], xh[0], xh[1], xh[2], xl[0], xl[1], xl[2],
         nh, nl, ones, ones]
    ).astype(np.float16)
    rhs = np.stack(
        [-2 * xh[0], -2 * xh[1], -2 * xh[2], -2 * xl[0], -2 * xl[1], -2 * xl[2],
         -2 * xh[0], -2 * xh[1], -2 * xh[2], ones, ones, nh, nl]
    ).astype(np.float16)
    return lhs, rhs


def _kd_order(pts):
    """Recursive median split along the widest dim -> permutation whose
    consecutive 128-row groups are compact KD leaves."""
    out = []

    def rec(ids):
        if len(ids) <= 128:
            out.append(ids)
            return
        p = pts[ids]
        dim = int(np.argmax(p.max(0) - p.min(0)))
        half = len(ids) // 2
        part = np.argpartition(p[:, dim], half)
        rec(ids[part[:half]])
        rec(ids[part[half:]])

    rec(np.arange(len(pts)))
    return np.concatenate(out)


def _candidates(sorted_q, other):
    """Per 128-row leaf of sorted_q: indices of the W other-set points
    nearest to the leaf AABB, and the certification radius B_g."""
    nl = sorted_q.shape[0] // 128
    leaves = sorted_q.reshape(nl, 128, D)
    lo = leaves.min(axis=1)
    hi = leaves.max(axis=1)
    d = np.maximum(
        np.maximum(lo[:, None, :] - other[None, :, :],
                   other[None, :, :] - hi[:, None, :]),
        0.0,
    )
    bd = np.sqrt((d * d).sum(-1))               # [nl, n_other]
    part = np.argpartition(bd, W, axis=1)
    cand = part[:, :W]                          # [nl, W]
    Bg = np.take_along_axis(bd, part[:, W : W + 1], axis=1)[:, 0]
    return cand, Bg


def _plane(lhs13, rhs13_other, leaf_ids, cand):
    """One PE-row-group input plane: per direction, 4 x [lhs 128 | rhs W]
    column groups for this plane's leaves, direction-1 then direction-2
    halves supplied by the caller via (leaf_ids, cand) lists."""
    pieces = []
    for (lhs_src, rhs_src, lids, cnd) in zip(
        lhs13, rhs13_other, leaf_ids, cand
    ):
        for b in lids:
            pieces.append(lhs_src[:, b * 128 : (b + 1) * 128])
            pieces.append(rhs_src[:, cnd[b]])
    return np.ascontiguousarray(
        np.concatenate(pieces, axis=1).astype(np.float16)
    )


def _run_device(s1, s2, trace=False):
    """Returns (d1, d2, res): exact per-row NN distances (KD-sorted order)
    for both directions, plus the device result object."""
    from concourse.bass_utils import run_bass_kernel_spmd

    nc = _get_program()
    s1_64 = np.asarray(s1, dtype=np.float64)
    s2_64 = np.asarray(s2, dtype=np.float64)

    perm1 = _kd_order(s1_64)
    perm2 = _kd_order(s2_64)
    s1s = s1_64[perm1]
    s2s = s2_64[perm2]

    cand1, B1 = _candidates(s1s, s2_64)   # dir 1->2
    cand2, B2 = _candidates(s2s, s1_64)   # dir 2->1

    lhs1_13, _ = _aug_stacks(s1s)
    lhs2_13, _ = _aug_stacks(s2s)
    _, rhs2_13 = _aug_stacks(s2_64)
    _, rhs1_13 = _aug_stacks(s1_64)

    in_maps = []
    for r in range(N_CORES):
        base = r * BLOCKS
        # plane 0: even local leaves (PE row group 0); plane 1: odd
        ev = [base + b for b in range(0, BLOCKS, 2)]
        od = [base + b for b in range(1, BLOCKS, 2)]
        in_maps.append(
            {
                "in0": _plane(
                    [lhs1_13, lhs2_13],
                    [rhs2_13, rhs1_13],
                    [ev, ev],
                    [cand1, cand2],
                ),
                "in1": _plane(
                    [lhs1_13, lhs2_13],
                    [rhs2_13, rhs1_13],
                    [od, od],
                    [cand1, cand2],
                ),
            }
        )

    last_err = None
    for _attempt in range(3):
        try:
            res = run_bass_kernel_spmd(nc, in_maps, list(range(N_CORES)), trace=trace)
            break
        except Exception as e:
            last_err = e
    else:
        raise last_err

    # out[:, 8o + 4g + 2j + i] = row-min^2 of leaf (4g + 2j + i%...):
    # within a dir: col order is [pair0: even leaf, odd leaf][pair1: ...]
    # i.e. col c (0..7) -> pair c//2, plane c%2 -> local leaf 2*(c//2)+(c%2)
    # which equals c.  So col c of direction o = local leaf c.
    # device rm slot order within each 4-leaf group is [0, 2, 1, 3]
    # (bank-A chunks then bank-B chunks); un-permute to leaf order.
    SLOT = [0, 2, 1, 3, 4, 6, 5, 7]

    def gather(o):
        outs = []
        for r in range(N_CORES):
            block = res.results[r]["out"][:, 8 * o : 8 * o + 8]  # [128, 8]
            outs.append(block[:, SLOT].T.reshape(-1))            # leaf-major
        return np.concatenate(outs)

    d1min = gather(0)
    d2min = gather(1)

    def finalize(dmin2, sorted_q, other, Bg):
        d = np.sqrt(np.maximum(dmin2, 0.0).astype(np.float64))
        bound = np.repeat(Bg, 128)
        bad = np.nonzero(d * (1.0 + 1e-3) + 1e-6 > bound)[0]
        if len(bad):
            diff = sorted_q[bad, None, :] - other[None, :, :]
            d[bad] = np.sqrt((diff * diff).sum(-1).min(axis=1))
        return d

    d1 = finalize(d1min, s1s, s2_64, B1)
    d2 = finalize(d2min, s2s, s1_64, B2)
    return d1, d2, res


def kernel(set1, set2, hausdorff=0, w_set1_set2=1, w_set2_set1=1, n_outputs=1):
    s1 = np.ascontiguousarray(np.asarray(set1, dtype=np.float32))
    s2 = np.ascontiguousarray(np.asarray(set2, dtype=np.float32))
    assert s1.shape == (N, D) and s2.shape == (M, D), (s1.shape, s2.shape)
    hausdorff = int(np.asarray(hausdorff))
    w12 = int(np.asarray(w_set1_set2))
    w21 = int(np.asarray(w_set2_set1))
    n_outputs = int(np.asarray(n_outputs))

    d1, d2, _ = _run_device(s1, s2)

    reduce = np.mean if hausdorff == 0 else np.max
    t12 = np.float32(reduce(d1)) if w12 != 0 else np.float32(0.0)
    t21 = np.float32(reduce(d2)) if w21 != 0 else np.float32(0.0)

    if n_outputs == 1:
        return np.float32(t12 + t21)
    return (t12, t21)


# revision 3
# speedup vs baseline: 1.1593x; 1.1593x over previous
"""Averaged Hausdorff loss kernel for 8 Trainium2 NeuronCores — v2.

Same exact windowed nearest-neighbor formulation as v1 (KD leaves of 128
rows, host-certified window, host fixup of uncertified rows), with the
device program restructured around the measured bottlenecks:

- W=160 candidates per leaf (was 384): one [13,128]x[13,160] matmul per
  leaf, 16 total per core, alternating PE row groups 0/32.  ~12% of rows
  fail certification at W=160 on N(0,1)^3 data and are recomputed on the
  host (exact either way; certification makes this exact for any input).
- PSUM tile per 2-pair group = 2 banks: bank A holds the row-group-0
  chunks of both pairs (offsets 0/256), bank B the row-group-1 chunks,
  so the two concurrent matmul streams never share a bank (sharing one
  faults) and the 4 chunks sit at uniform stride 256 for rank-3 APs.
  ScalarE stages the second 80 of each 160-chunk to SBUF; the fused
  dual-port DVE min-scan (MINSCAN_SEG_ANT) reduces 80 PSUM + 80 SBUF
  element pairs per leaf at 2 fp32/cycle.
- Inputs ride all 3 DMA queues: sync/scalar (HWDGE) carry direction 1
  split early (pairs 0-1) + rest, gpsimd (SWDGE) carries direction 2 in
  parallel.  Within a group both bank-A matmuls are emitted before the
  bank-B ones (plane-1 data lands ~0.9us after plane 0).  Outputs are 4
  small DMAs alternating sync/scalar so the last waits only on the
  final DVE op.  PE warm-up was tried and removed: this part pins the
  PE at ~1.2 GHz regardless of HAM activity.
"""

import sys

sys.path.insert(0, "/opt/trn_rl_repo")

import numpy as np

N_CORES = 8
N = 8192          # set1 rows
M = 8192          # set2 rows
D = 3
ROWS_PER_CORE = N // N_CORES          # 1024
BLOCKS = ROWS_PER_CORE // 128         # 8 KD leaves per core per direction
NLEAF = N // 128                      # 64 leaves total per direction
W = 160                               # candidate window per leaf
K = 13                                # augmented contraction dim
FP32_MAX = 3.4e38
PAIR_COLS = 128 + W                   # lhs + rhs columns per leaf
DIR_COLS = (BLOCKS // 2) * PAIR_COLS  # 4 pairs per plane per direction
TOT = 2 * DIR_COLS                    # plane width
EARLY = 2 * PAIR_COLS                 # first two pairs of direction 1
N_WARM = 12                           # PE warm-up matmuls

_compiled = None


def _register_minseg():
    """Fused segmented DVE op (identical to v1): per-segment running
    min-scan over min(in0[p,..,k], in1[p,..,k]), reseeded from s0 at every
    innermost-dim (SUB_DIM) boundary.  Written through an AP whose
    innermost dim is stride-0, so each segment's destination cell ends
    with that segment's total min."""
    from concourse import dve_ops
    from concourse import dve_spec as ds
    from concourse.dve_uop import DveOpSpec

    def _ref(in0, in1, c0, c1, c2):
        b = np.minimum(in0.astype(np.float32), in1.astype(np.float32))
        P = b.shape[0]
        flat = b.reshape(P, -1, b.shape[-1])
        init = np.full((P, flat.shape[1], 1), c0, np.float32)
        out = np.minimum.accumulate(
            np.concatenate([init, flat], axis=-1), axis=-1
        )[:, :, 1:]
        return out.reshape(b.shape)

    name = "MINSCAN_SEG_ANT"
    if name in dve_ops._SUB_OPCODE_FOR_NAME:
        return next(op for op in dve_ops.OPS if op.name == name)

    body = ds.scan(ds.AluOp.MIN, ds.minn(ds.Src0, ds.Src1), init=ds.C0)
    spec = ds.Spec(body=body, reference=_ref)

    def lower_seg(ver):
        n_lanes, n_stages = ds.N_LANES[ver], ds.N_STAGES[ver]
        ds._validate_body(spec, ver)
        sp = ds._hoist_stream_invariant_ops(spec)
        scans = ds._collect(sp.body, ds.Scan)
        latches = ds._collect(sp.body, ds.Latch)
        placement = ds._build_placement(sp, scans, n_stages, n_lanes)
        states = ds._build_state_machine(sp, scans, latches, placement)
        assert len(states) == 2, states  # [seed, steady]
        seed, steady = states
        d = placement.node_stage[scans[0]]
        steady2 = ds._State(
            placement=placement,
            consume=steady.consume,
            trigger=(
                ds.Trigger.SRC_TENSOR_DONE,
                ds.Trigger.SUB_DIM_DONE,
                ds.Trigger.NONE,
            ),
            next=(0, 2, 0),
        )
        step = ds._State(
            placement=placement,
            consume=steady.consume,
            overrides={d: ds._Stage(scans[0].op, ds.C0, scans[0].expr)},
            trigger=(
                ds.Trigger.SRC_TENSOR_DONE,
                ds.Trigger.SUB_DIM_DONE,
                ds.Trigger.COUNT,
            ),
            next=(0, 2, 1),
            repeat=1,
        )
        uops = [ds._assemble(s) for s in (seed, steady2, step)]
        for u in uops:
            u.validate(ver)
        return uops

    op = dve_ops.DveOp(name, spec, subdim=True, uops_sha={})
    dve_ops.OPS.append(op)
    dve_ops._SUB_OPCODE_FOR_NAME[name] = (
        dve_ops._CUSTOM_DVE_ROW_BASE + len(dve_ops.OPS) - 1
    )
    assert dve_ops._SUB_OPCODE_FOR_NAME[name] < 0x20
    dve_ops.CUSTOM_DVE_SPECS[name] = spec
    for ver in ("v3", "v4"):
        compiled = DveOpSpec(
            name=name,
            opcode=dve_ops.get_dve_sub_opcode(name),
            uops=lower_seg(ver),
            rd1_en=True,
        )
        op.uops_sha[ver] = compiled.sha(ver)
        dve_ops._COMPILE_CACHE[(name, ver)] = compiled
    return op


def _build_program():
    from concourse import bacc, mybir

    minseg = _register_minseg()

    nc = bacc.Bacc("TRN2", target_bir_lowering=False, debug=False)
    f32 = mybir.dt.float32
    f16 = mybir.dt.float16

    KR = 32 + K   # SBUF operand stack height (rows 0..12 and 32..44)
    H = W // 2    # half-chunk length for the dual-port scan
    RA = DIR_COLS - EARLY

    in0_d = nc.dram_tensor("in0", [K, TOT], f16, kind="ExternalInput")
    in1_d = nc.dram_tensor("in1", [K, TOT], f16, kind="ExternalInput")
    out_d = nc.dram_tensor("out", [128, 16], f32, kind="ExternalOutput")

    # Raw bass (no TileContext): every dependency is a hand-placed
    # semaphore.  4 PSUM groups x 2 banks = all 8 banks, no reuse, so
    # the only hazards are the RAW chains below.  No final barrier: each
    # engine's stream simply ends, and the compiler-injected teardown
    # does its own wait-for-all before the semaphore resets.
    t0 = [
        nc.alloc_sbuf_tensor("t0e", [KR, EARLY], f16),
        nc.alloc_sbuf_tensor("t0ra", [KR, RA], f16),
        nc.alloc_sbuf_tensor("t0rb", [KR, DIR_COLS], f16),
    ]
    t1 = [
        nc.alloc_sbuf_tensor("t1e", [KR, EARLY], f16),
        nc.alloc_sbuf_tensor("t1ra", [KR, RA], f16),
        nc.alloc_sbuf_tensor("t1rb", [KR, DIR_COLS], f16),
    ]
    sc = [nc.alloc_sbuf_tensor(f"sc{g}", [128, 4, H], f32) for g in range(4)]
    rm = [nc.alloc_sbuf_tensor(f"rm{g}", [128, 4], f32) for g in range(4)]
    pk = [
        nc.alloc_psum_tensor(f"pk{g}", [128, 4, 256], f32) for g in range(4)
    ]

    se0 = nc.alloc_semaphore("se0")
    se1 = nc.alloc_semaphore("se1")
    sra0 = nc.alloc_semaphore("sra0")
    sra1 = nc.alloc_semaphore("sra1")
    srb0 = nc.alloc_semaphore("srb0")
    srb1 = nc.alloc_semaphore("srb1")
    spe = nc.alloc_semaphore("spe")
    ssc = nc.alloc_semaphore("ssc")
    sdve = nc.alloc_semaphore("sdve")
    sout = nc.alloc_semaphore("sout")

    # --- sync: plane-0 inputs, then out DMAs for groups 0/2 ---
    nc.sync.dma_start(t0[0].ap()[0:K, :], in0_d.ap()[:, 0:EARLY]).then_inc(
        se0, 16
    )
    nc.sync.dma_start(
        t0[1].ap()[0:K, :], in0_d.ap()[:, EARLY:DIR_COLS]
    ).then_inc(sra0, 16)

    # --- scalar: plane-1 inputs, act table, SC copies, outs 1/3 ---
    nc.scalar.dma_start(
        t1[0].ap()[32 : 32 + K, :], in1_d.ap()[:, 0:EARLY]
    ).then_inc(se1, 16)
    nc.scalar.dma_start(
        t1[1].ap()[32 : 32 + K, :], in1_d.ap()[:, EARLY:DIR_COLS]
    ).then_inc(sra1, 16)
    # Activation-table load: ends up at the scalar stream head whatever
    # we do (its table-fetch DMA contends with the t1e issue, +0.7us on
    # t1e's data — attempts to pin it later did not survive compile).
    _atl = mybir.InstLoadActFuncSet(
        name="act_tbl_preload", ins=[], outs=[], act_func_set_id=0
    )
    _atl.engine = nc.scalar.engine
    nc.scalar.add_instruction(_atl)

    # --- gpsimd: direction-2 planes (SWDGE, parallel issue) ---
    nc.gpsimd.dma_start(
        t0[2].ap()[0:K, :], in0_d.ap()[:, DIR_COLS:TOT]
    ).then_inc(srb0, 16)
    nc.gpsimd.dma_start(
        t1[2].ap()[32 : 32 + K, :], in1_d.ap()[:, DIR_COLS:TOT]
    ).then_inc(srb1, 16)

    def cols(o, p):
        if o == 1:
            return p * PAIR_COLS, 2
        if p < 2:
            return p * PAIR_COLS, 0
        return p * PAIR_COLS - EARLY, 1

    # --- tensor: 16 matmuls, bank-A chunks before bank-B per group ---
    in_sems = {(0, 0): se0, (0, 1): se1, (1, 0): sra0, (1, 1): sra1,
               (2, 0): srb0, (2, 1): srb1}
    waited = set()
    npe = 0
    for o in range(2):
        for g in range(2):
            gi = 2 * o + g
            for c in range(2):
                t = (t0, t1)[c]
                rows = slice(32 * c, 32 * c + K)
                for j in range(2):
                    p = 2 * g + j
                    off, piece = cols(o, p)
                    if (piece, c) not in waited:
                        waited.add((piece, c))
                        nc.tensor.wait_ge(in_sems[(piece, c)], 16)
                    l0 = slice(off, off + 128)
                    r0 = slice(off + 128, off + 128 + W)
                    ap = t[piece].ap()
                    nc.tensor.matmul(
                        pk[gi].ap()[:, 2 * c + j, 0:W], ap[rows, l0],
                        ap[rows, r0],
                    ).then_inc(spe, 1)
                    npe += 1
    assert npe == 16

    # --- scalar: SC copies (second halves -> SBUF) ---
    for gi in range(4):
        nc.scalar.wait_ge(spe, 4 * (gi + 1))
        nc.scalar.copy(sc[gi].ap(), pk[gi].ap()[:, :, H:W]).then_inc(ssc, 1)

    # --- vector: fused dual-port segmented min-scans ---
    for gi in range(4):
        nc.vector.wait_ge(ssc, gi + 1)
        nc.vector._custom_dve(
            minseg,
            out=rm[gi].ap().broadcast_to((128, 4, H)),
            in0=pk[gi].ap()[:, :, 0:H],
            in1=sc[gi].ap(),
            s0=FP32_MAX,
        ).then_inc(sdve, 1)

    # --- out DMAs: groups 0/2 on sync, 1/3 on scalar ---
    for gi in range(4):
        eng = nc.sync if gi % 2 == 0 else nc.scalar
        eng.wait_ge(sdve, gi + 1)
        eng.dma_start(
            out_d.ap()[:, 4 * gi : 4 * gi + 4], rm[gi].ap()
        ).then_inc(sout, 16)

    # hold the program open until every output landed in DRAM
    nc.sync.wait_ge(sout, 64)

    nc.compile()
    return nc


def _get_program():
    global _compiled
    if _compiled is None:
        _compiled = _build_program()
    return _compiled


def _split16(v):
    """fp64 vector -> (hi, lo) fp16 with v ~= hi + lo to ~2^-22 rel."""
    hi = v.astype(np.float16)
    lo = (v - hi.astype(np.float64)).astype(np.float16)
    return hi.astype(np.float64), lo.astype(np.float64)


def _aug_stacks(s64):
    """[n, 3] fp64 -> ([13, n] lhs stack, [13, n] rhs stack) fp16."""
    n = (s64 * s64).sum(axis=1)
    ones = np.ones(s64.shape[0], dtype=np.float64)
    xh = [None] * D
    xl = [None] * D
    for d in range(D):
        xh[d], xl[d] = _split16(s64[:, d])
    nh, nl = _split16(n)
    lhs = np.stack(
        [xh[0], xh[1], xh[2], xh[0], xh[1], xh[2], xl[0], xl[1], xl[2],
         nh, nl, ones, ones]
    ).astype(np.float16)
    rhs = np.stack(
        [-2 * xh[0], -2 * xh[1], -2 * xh[2], -2 * xl[0], -2 * xl[1], -2 * xl[2],
         -2 * xh[0], -2 * xh[1], -2 * xh[2], ones, ones, nh, nl]
    ).astype(np.float16)
    return lhs, rhs


def _kd_order(pts):
    """Recursive median split along the widest dim -> permutation whose
    consecutive 128-row groups are compact KD leaves."""
    out = []

    def rec(ids):
        if len(ids) <= 128:
            out.append(ids)
            return
        p = pts[ids]
        dim = int(np.argmax(p.max(0) - p.min(0)))
        half = len(ids) // 2
        part = np.argpartition(p[:, dim], half)
        rec(ids[part[:half]])
        rec(ids[part[half:]])

    rec(np.arange(len(pts)))
    return np.concatenate(out)


def _candidates(sorted_q, other):
    """Per 128-row leaf of sorted_q: indices of the W other-set points
    nearest to the leaf AABB, and the certification radius B_g."""
    nl = sorted_q.shape[0] // 128
    leaves = sorted_q.reshape(nl, 128, D)
    lo = leaves.min(axis=1)
    hi = leaves.max(axis=1)
    d = np.maximum(
        np.maximum(lo[:, None, :] - other[None, :, :],
                   other[None, :, :] - hi[:, None, :]),
        0.0,
    )
    bd = np.sqrt((d * d).sum(-1))               # [nl, n_other]
    part = np.argpartition(bd, W, axis=1)
    cand = part[:, :W]                          # [nl, W]
    Bg = np.take_along_axis(bd, part[:, W : W + 1], axis=1)[:, 0]
    return cand, Bg


def _plane(lhs13, rhs13_other, leaf_ids, cand):
    """One PE-row-group input plane: per direction, 4 x [lhs 128 | rhs W]
    column groups for this plane's leaves, direction-1 then direction-2
    halves supplied by the caller via (leaf_ids, cand) lists."""
    pieces = []
    for (lhs_src, rhs_src, lids, cnd) in zip(
        lhs13, rhs13_other, leaf_ids, cand
    ):
        for b in lids:
            pieces.append(lhs_src[:, b * 128 : (b + 1) * 128])
            pieces.append(rhs_src[:, cnd[b]])
    return np.ascontiguousarray(
        np.concatenate(pieces, axis=1).astype(np.float16)
    )


def _run_device(s1, s2, trace=False):
    """Returns (d1, d2, res): exact per-row NN distances (KD-sorted order)
    for both directions, plus the device result object."""
    from concourse.bass_utils import run_bass_kernel_spmd

    nc = _get_program()
    s1_64 = np.asarray(s1, dtype=np.float64)
    s2_64 = np.asarray(s2, dtype=np.float64)

    perm1 = _kd_order(s1_64)
    perm2 = _kd_order(s2_64)
    s1s = s1_64[perm1]
    s2s = s2_64[perm2]

    cand1, B1 = _candidates(s1s, s2_64)   # dir 1->2
    cand2, B2 = _candidates(s2s, s1_64)   # dir 2->1

    lhs1_13, _ = _aug_stacks(s1s)
    lhs2_13, _ = _aug_stacks(s2s)
    _, rhs2_13 = _aug_stacks(s2_64)
    _, rhs1_13 = _aug_stacks(s1_64)

    in_maps = []
    for r in range(N_CORES):
        base = r * BLOCKS
        # plane 0: even local leaves (PE row group 0); plane 1: odd
        ev = [base + b for b in range(0, BLOCKS, 2)]
        od = [base + b for b in range(1, BLOCKS, 2)]
        in_maps.append(
            {
                "in0": _plane(
                    [lhs1_13, lhs2_13],
                    [rhs2_13, rhs1_13],
                    [ev, ev],
                    [cand1, cand2],
                ),
                "in1": _plane(
                    [lhs1_13, lhs2_13],
                    [rhs2_13, rhs1_13],
                    [od, od],
                    [cand1, cand2],
                ),
            }
        )

    last_err = None
    for _attempt in range(3):
        try:
            res = run_bass_kernel_spmd(nc, in_maps, list(range(N_CORES)), trace=trace)
            break
        except Exception as e:
            last_err = e
    else:
        raise last_err

    # out[:, 8o + 4g + 2j + i] = row-min^2 of leaf (4g + 2j + i%...):
    # within a dir: col order is [pair0: even leaf, odd leaf][pair1: ...]
    # i.e. col c (0..7) -> pair c//2, plane c%2 -> local leaf 2*(c//2)+(c%2)
    # which equals c.  So col c of direction o = local leaf c.
    # device rm slot order within each 4-leaf group is [0, 2, 1, 3]
    # (bank-A chunks then bank-B chunks); un-permute to leaf order.
    SLOT = [0, 2, 1, 3, 4, 6, 5, 7]

    def gather(o):
        outs = []
        for r in range(N_CORES):
            block = res.results[r]["out"][:, 8 * o : 8 * o + 8]  # [128, 8]
            outs.append(block[:, SLOT].T.reshape(-1))            # leaf-major
        return np.concatenate(outs)

    d1min = gather(0)
    d2min = gather(1)

    def finalize(dmin2, sorted_q, other, Bg):
        d = np.sqrt(np.maximum(dmin2, 0.0).astype(np.float64))
        bound = np.repeat(Bg, 128)
        bad = np.nonzero(d * (1.0 + 1e-3) + 1e-6 > bound)[0]
        if len(bad):
            diff = sorted_q[bad, None, :] - other[None, :, :]
            d[bad] = np.sqrt((diff * diff).sum(-1).min(axis=1))
        return d

    d1 = finalize(d1min, s1s, s2_64, B1)
    d2 = finalize(d2min, s2s, s1_64, B2)
    return d1, d2, res


def kernel(set1, set2, hausdorff=0, w_set1_set2=1, w_set2_set1=1, n_outputs=1):
    s1 = np.ascontiguousarray(np.asarray(set1, dtype=np.float32))
    s2 = np.ascontiguousarray(np.asarray(set2, dtype=np.float32))
    assert s1.shape == (N, D) and s2.shape == (M, D), (s1.shape, s2.shape)
    hausdorff = int(np.asarray(hausdorff))
    w12 = int(np.asarray(w_set1_set2))
    w21 = int(np.asarray(w_set2_set1))
    n_outputs = int(np.asarray(n_outputs))

    d1, d2, _ = _run_device(s1, s2)

    reduce = np.mean if hausdorff == 0 else np.max
    t12 = np.float32(reduce(d1)) if w12 != 0 else np.float32(0.0)
    t21 = np.float32(reduce(d2)) if w21 != 0 else np.float32(0.0)

    if n_outputs == 1:
        return np.float32(t12 + t21)
    return (t12, t21)


# revision 4
# speedup vs baseline: 1.1600x; 1.0006x over previous
"""Averaged Hausdorff loss kernel for 8 Trainium2 NeuronCores — v2.

Same exact windowed nearest-neighbor formulation as v1 (KD leaves of 128
rows, host-certified window, host fixup of uncertified rows), with the
device program restructured around the measured bottlenecks:

- W=160 candidates per leaf (was 384): one [13,128]x[13,160] matmul per
  leaf, 16 total per core, alternating PE row groups 0/32.  ~12% of rows
  fail certification at W=160 on N(0,1)^3 data and are recomputed on the
  host (exact either way; certification makes this exact for any input).
- PSUM tile per 2-pair group = 2 banks: bank A holds the row-group-0
  chunks of both pairs (offsets 0/256), bank B the row-group-1 chunks,
  so the two concurrent matmul streams never share a bank (sharing one
  faults) and the 4 chunks sit at uniform stride 256 for rank-3 APs.
  ScalarE stages the second 80 of each 160-chunk to SBUF; the fused
  dual-port DVE min-scan (MINSCAN_SEG_ANT) reduces 80 PSUM + 80 SBUF
  element pairs per leaf at 2 fp32/cycle.
- Inputs ride all 3 DMA queues: sync/scalar (HWDGE) carry direction 1
  split early (pairs 0-1) + rest, gpsimd (SWDGE) carries direction 2 in
  parallel.  Within a group both bank-A matmuls are emitted before the
  bank-B ones (plane-1 data lands ~0.9us after plane 0).  Outputs are 4
  small DMAs alternating sync/scalar so the last waits only on the
  final DVE op.  PE warm-up was tried and removed: this part pins the
  PE at ~1.2 GHz regardless of HAM activity.
"""

import sys

sys.path.insert(0, "/opt/trn_rl_repo")

import numpy as np

N_CORES = 8
N = 8192          # set1 rows
M = 8192          # set2 rows
D = 3
ROWS_PER_CORE = N // N_CORES          # 1024
BLOCKS = ROWS_PER_CORE // 128         # 8 KD leaves per core per direction
NLEAF = N // 128                      # 64 leaves total per direction
W = 160                               # candidate window per leaf
K = 13                                # augmented contraction dim
FP32_MAX = 3.4e38
PAIR_COLS = 128 + W                   # lhs + rhs columns per leaf
DIR_COLS = (BLOCKS // 2) * PAIR_COLS  # 4 pairs per plane per direction
TOT = 2 * DIR_COLS                    # plane width
EARLY = 2 * PAIR_COLS                 # first two pairs of direction 1
N_WARM = 12                           # PE warm-up matmuls

_compiled = None


def _register_minseg():
    """Fused segmented DVE op (identical to v1): per-segment running
    min-scan over min(in0[p,..,k], in1[p,..,k]), reseeded from s0 at every
    innermost-dim (SUB_DIM) boundary.  Written through an AP whose
    innermost dim is stride-0, so each segment's destination cell ends
    with that segment's total min."""
    from concourse import dve_ops
    from concourse import dve_spec as ds
    from concourse.dve_uop import DveOpSpec

    def _ref(in0, in1, c0, c1, c2):
        b = np.minimum(in0.astype(np.float32), in1.astype(np.float32))
        P = b.shape[0]
        flat = b.reshape(P, -1, b.shape[-1])
        init = np.full((P, flat.shape[1], 1), c0, np.float32)
        out = np.minimum.accumulate(
            np.concatenate([init, flat], axis=-1), axis=-1
        )[:, :, 1:]
        return out.reshape(b.shape)

    name = "MINSCAN_SEG_ANT"
    if name in dve_ops._SUB_OPCODE_FOR_NAME:
        return next(op for op in dve_ops.OPS if op.name == name)

    body = ds.scan(ds.AluOp.MIN, ds.minn(ds.Src0, ds.Src1), init=ds.C0)
    spec = ds.Spec(body=body, reference=_ref)

    def lower_seg(ver):
        n_lanes, n_stages = ds.N_LANES[ver], ds.N_STAGES[ver]
        ds._validate_body(spec, ver)
        sp = ds._hoist_stream_invariant_ops(spec)
        scans = ds._collect(sp.body, ds.Scan)
        latches = ds._collect(sp.body, ds.Latch)
        placement = ds._build_placement(sp, scans, n_stages, n_lanes)
        states = ds._build_state_machine(sp, scans, latches, placement)
        assert len(states) == 2, states  # [seed, steady]
        seed, steady = states
        d = placement.node_stage[scans[0]]
        steady2 = ds._State(
            placement=placement,
            consume=steady.consume,
            trigger=(
                ds.Trigger.SRC_TENSOR_DONE,
                ds.Trigger.SUB_DIM_DONE,
                ds.Trigger.NONE,
            ),
            next=(0, 2, 0),
        )
        step = ds._State(
            placement=placement,
            consume=steady.consume,
            overrides={d: ds._Stage(scans[0].op, ds.C0, scans[0].expr)},
            trigger=(
                ds.Trigger.SRC_TENSOR_DONE,
                ds.Trigger.SUB_DIM_DONE,
                ds.Trigger.COUNT,
            ),
            next=(0, 2, 1),
            repeat=1,
        )
        uops = [ds._assemble(s) for s in (seed, steady2, step)]
        for u in uops:
            u.validate(ver)
        return uops

    op = dve_ops.DveOp(name, spec, subdim=True, uops_sha={})
    dve_ops.OPS.append(op)
    dve_ops._SUB_OPCODE_FOR_NAME[name] = (
        dve_ops._CUSTOM_DVE_ROW_BASE + len(dve_ops.OPS) - 1
    )
    assert dve_ops._SUB_OPCODE_FOR_NAME[name] < 0x20
    dve_ops.CUSTOM_DVE_SPECS[name] = spec
    for ver in ("v3", "v4"):
        compiled = DveOpSpec(
            name=name,
            opcode=dve_ops.get_dve_sub_opcode(name),
            uops=lower_seg(ver),
            rd1_en=True,
        )
        op.uops_sha[ver] = compiled.sha(ver)
        dve_ops._COMPILE_CACHE[(name, ver)] = compiled
    return op


def _build_program():
    from concourse import bacc, mybir

    minseg = _register_minseg()

    nc = bacc.Bacc("TRN2", target_bir_lowering=False, debug=False)
    f32 = mybir.dt.float32
    f16 = mybir.dt.float16

    KR = 32 + K   # SBUF operand stack height (rows 0..12 and 32..44)
    H = W // 2    # half-chunk length for the dual-port scan
    RA = DIR_COLS - EARLY

    in0_d = nc.dram_tensor("in0", [K, TOT], f16, kind="ExternalInput")
    in1_d = nc.dram_tensor("in1", [K, TOT], f16, kind="ExternalInput")
    out_d = nc.dram_tensor("out", [128, 16], f32, kind="ExternalOutput")

    # Raw bass (no TileContext): every dependency is a hand-placed
    # semaphore.  4 PSUM groups x 2 banks = all 8 banks, no reuse, so
    # the only hazards are the RAW chains below.  No final barrier: each
    # engine's stream simply ends, and the compiler-injected teardown
    # does its own wait-for-all before the semaphore resets.
    t0 = [
        nc.alloc_sbuf_tensor("t0e", [KR, EARLY], f16),
        nc.alloc_sbuf_tensor("t0ra", [KR, RA], f16),
        nc.alloc_sbuf_tensor("t0rb", [KR, DIR_COLS], f16),
    ]
    t1 = [
        nc.alloc_sbuf_tensor("t1e", [KR, EARLY], f16),
        nc.alloc_sbuf_tensor("t1ra", [KR, RA], f16),
        nc.alloc_sbuf_tensor("t1rb", [KR, DIR_COLS], f16),
    ]
    sc = [nc.alloc_sbuf_tensor(f"sc{g}", [128, 4, H], f32) for g in range(4)]
    rm = [nc.alloc_sbuf_tensor(f"rm{g}", [128, 4], f32) for g in range(4)]
    pk = [
        nc.alloc_psum_tensor(f"pk{g}", [128, 4, 256], f32) for g in range(4)
    ]

    se0 = nc.alloc_semaphore("se0")
    se1 = nc.alloc_semaphore("se1")
    sra0 = nc.alloc_semaphore("sra0")
    sra1 = nc.alloc_semaphore("sra1")
    srb0 = nc.alloc_semaphore("srb0")
    srb1 = nc.alloc_semaphore("srb1")
    spe = nc.alloc_semaphore("spe")
    ssc = nc.alloc_semaphore("ssc")
    sdve = nc.alloc_semaphore("sdve")
    sout = nc.alloc_semaphore("sout")

    # --- sync: plane-0 inputs, then out DMAs for groups 0/2 ---
    nc.sync.dma_start(t0[0].ap()[0:K, :], in0_d.ap()[:, 0:EARLY]).then_inc(
        se0, 16
    )
    nc.sync.dma_start(
        t0[1].ap()[0:K, :], in0_d.ap()[:, EARLY:DIR_COLS]
    ).then_inc(sra0, 16)

    # --- scalar: plane-1 inputs, act table, SC copies, outs 1/3 ---
    nc.scalar.dma_start(
        t1[0].ap()[32 : 32 + K, :], in1_d.ap()[:, 0:EARLY]
    ).then_inc(se1, 16)
    nc.scalar.dma_start(
        t1[1].ap()[32 : 32 + K, :], in1_d.ap()[:, EARLY:DIR_COLS]
    ).then_inc(sra1, 16)
    # Activation-table load: ends up at the scalar stream head whatever
    # we do (its table-fetch DMA contends with the t1e issue, +0.7us on
    # t1e's data — attempts to pin it later did not survive compile).
    _atl = mybir.InstLoadActFuncSet(
        name="act_tbl_preload", ins=[], outs=[], act_func_set_id=0
    )
    _atl.engine = nc.scalar.engine
    nc.scalar.add_instruction(_atl)

    # --- gpsimd: direction-2 planes (SWDGE, parallel issue) ---
    nc.gpsimd.dma_start(
        t0[2].ap()[0:K, :], in0_d.ap()[:, DIR_COLS:TOT]
    ).then_inc(srb0, 16)
    nc.gpsimd.dma_start(
        t1[2].ap()[32 : 32 + K, :], in1_d.ap()[:, DIR_COLS:TOT]
    ).then_inc(srb1, 16)

    def cols(o, p):
        if o == 1:
            return p * PAIR_COLS, 2
        if p < 2:
            return p * PAIR_COLS, 0
        return p * PAIR_COLS - EARLY, 1

    # --- tensor: 16 matmuls, bank-A chunks before bank-B per group ---
    in_sems = {(0, 0): se0, (0, 1): se1, (1, 0): sra0, (1, 1): sra1,
               (2, 0): srb0, (2, 1): srb1}
    waited = set()
    npe = 0
    for o in range(2):
        for g in range(2):
            gi = 2 * o + g
            for c in range(2):
                t = (t0, t1)[c]
                rows = slice(32 * c, 32 * c + K)
                for j in range(2):
                    p = 2 * g + j
                    off, piece = cols(o, p)
                    if (piece, c) not in waited:
                        waited.add((piece, c))
                        nc.tensor.wait_ge(in_sems[(piece, c)], 16)
                    l0 = slice(off, off + 128)
                    r0 = slice(off + 128, off + 128 + W)
                    ap = t[piece].ap()
                    nc.tensor.matmul(
                        pk[gi].ap()[:, 2 * c + j, 0:W], ap[rows, l0],
                        ap[rows, r0],
                    ).then_inc(spe, 1)
                    npe += 1
    assert npe == 16

    # --- scalar: SC copies (second halves -> SBUF) ---
    for gi in range(4):
        nc.scalar.wait_ge(spe, 4 * (gi + 1))
        nc.scalar.copy(sc[gi].ap(), pk[gi].ap()[:, :, H:W]).then_inc(ssc, 1)

    # --- vector: fused dual-port segmented min-scans ---
    for gi in range(4):
        nc.vector.wait_ge(ssc, gi + 1)
        nc.vector._custom_dve(
            minseg,
            out=rm[gi].ap().broadcast_to((128, 4, H)),
            in0=pk[gi].ap()[:, :, 0:H],
            in1=sc[gi].ap(),
            s0=FP32_MAX,
        ).then_inc(sdve, 1)

    # --- out DMAs: groups 0/2 on sync, 1/3 on scalar ---
    for gi in range(4):
        eng = nc.sync if gi % 2 == 0 else nc.scalar
        eng.wait_ge(sdve, gi + 1)
        eng.dma_start(
            out_d.ap()[:, 4 * gi : 4 * gi + 4], rm[gi].ap()
        ).then_inc(sout, 16)

    # No final out-completion wait: the compiler-injected teardown
    # drains the DGE rings itself (overlapped with the slow per-engine
    # semaphore resets), so outputs still land before NEFF completion.
    nc.compile()
    return nc


def _get_program():
    global _compiled
    if _compiled is None:
        _compiled = _build_program()
    return _compiled


def _split16(v):
    """fp64 vector -> (hi, lo) fp16 with v ~= hi + lo to ~2^-22 rel."""
    hi = v.astype(np.float16)
    lo = (v - hi.astype(np.float64)).astype(np.float16)
    return hi.astype(np.float64), lo.astype(np.float64)


def _aug_stacks(s64):
    """[n, 3] fp64 -> ([13, n] lhs stack, [13, n] rhs stack) fp16."""
    n = (s64 * s64).sum(axis=1)
    ones = np.ones(s64.shape[0], dtype=np.float64)
    xh = [None] * D
    xl = [None] * D
    for d in range(D):
        xh[d], xl[d] = _split16(s64[:, d])
    nh, nl = _split16(n)
    lhs = np.stack(
        [xh[0], xh[1], xh[2], xh[0], xh[1], xh[2], xl[0], xl[1], xl[2],
         nh, nl, ones, ones]
    ).astype(np.float16)
    rhs = np.stack(
        [-2 * xh[0], -2 * xh[1], -2 * xh[2], -2 * xl[0], -2 * xl[1], -2 * xl[2],
         -2 * xh[0], -2 * xh[1], -2 * xh[2], ones, ones, nh, nl]
    ).astype(np.float16)
    return lhs, rhs


def _kd_order(pts):
    """Recursive median split along the widest dim -> permutation whose
    consecutive 128-row groups are compact KD leaves."""
    out = []

    def rec(ids):
        if len(ids) <= 128:
            out.append(ids)
            return
        p = pts[ids]
        dim = int(np.argmax(p.max(0) - p.min(0)))
        half = len(ids) // 2
        part = np.argpartition(p[:, dim], half)
        rec(ids[part[:half]])
        rec(ids[part[half:]])

    rec(np.arange(len(pts)))
    return np.concatenate(out)


def _candidates(sorted_q, other):
    """Per 128-row leaf of sorted_q: indices of the W other-set points
    nearest to the leaf AABB, and the certification radius B_g."""
    nl = sorted_q.shape[0] // 128
    leaves = sorted_q.reshape(nl, 128, D)
    lo = leaves.min(axis=1)
    hi = leaves.max(axis=1)
    d = np.maximum(
        np.maximum(lo[:, None, :] - other[None, :, :],
                   other[None, :, :] - hi[:, None, :]),
        0.0,
    )
    bd = np.sqrt((d * d).sum(-1))               # [nl, n_other]
    part = np.argpartition(bd, W, axis=1)
    cand = part[:, :W]                          # [nl, W]
    Bg = np.take_along_axis(bd, part[:, W : W + 1], axis=1)[:, 0]
    return cand, Bg


def _plane(lhs13, rhs13_other, leaf_ids, cand):
    """One PE-row-group input plane: per direction, 4 x [lhs 128 | rhs W]
    column groups for this plane's leaves, direction-1 then direction-2
    halves supplied by the caller via (leaf_ids, cand) lists."""
    pieces = []
    for (lhs_src, rhs_src, lids, cnd) in zip(
        lhs13, rhs13_other, leaf_ids, cand
    ):
        for b in lids:
            pieces.append(lhs_src[:, b * 128 : (b + 1) * 128])
            pieces.append(rhs_src[:, cnd[b]])
    return np.ascontiguousarray(
        np.concatenate(pieces, axis=1).astype(np.float16)
    )


def _run_device(s1, s2, trace=False):
    """Returns (d1, d2, res): exact per-row NN distances (KD-sorted order)
    for both directions, plus the device result object."""
    from concourse.bass_utils import run_bass_kernel_spmd

    nc = _get_program()
    s1_64 = np.asarray(s1, dtype=np.float64)
    s2_64 = np.asarray(s2, dtype=np.float64)

    perm1 = _kd_order(s1_64)
    perm2 = _kd_order(s2_64)
    s1s = s1_64[perm1]
    s2s = s2_64[perm2]

    cand1, B1 = _candidates(s1s, s2_64)   # dir 1->2
    cand2, B2 = _candidates(s2s, s1_64)   # dir 2->1

    lhs1_13, _ = _aug_stacks(s1s)
    lhs2_13, _ = _aug_stacks(s2s)
    _, rhs2_13 = _aug_stacks(s2_64)
    _, rhs1_13 = _aug_stacks(s1_64)

    in_maps = []
    for r in range(N_CORES):
        base = r * BLOCKS
        # plane 0: even local leaves (PE row group 0); plane 1: odd
        ev = [base + b for b in range(0, BLOCKS, 2)]
        od = [base + b for b in range(1, BLOCKS, 2)]
        in_maps.append(
            {
                "in0": _plane(
                    [lhs1_13, lhs2_13],
                    [rhs2_13, rhs1_13],
                    [ev, ev],
                    [cand1, cand2],
                ),
                "in1": _plane(
                    [lhs1_13, lhs2_13],
                    [rhs2_13, rhs1_13],
                    [od, od],
                    [cand1, cand2],
                ),
            }
        )

    last_err = None
    for _attempt in range(3):
        try:
            res = run_bass_kernel_spmd(nc, in_maps, list(range(N_CORES)), trace=trace)
            break
        except Exception as e:
            last_err = e
    else:
        raise last_err

    # out[:, 8o + 4g + 2j + i] = row-min^2 of leaf (4g + 2j + i%...):
    # within a dir: col order is [pair0: even leaf, odd leaf][pair1: ...]
    # i.e. col c (0..7) -> pair c//2, plane c%2 -> local leaf 2*(c//2)+(c%2)
    # which equals c.  So col c of direction o = local leaf c.
    # device rm slot order within each 4-leaf group is [0, 2, 1, 3]
    # (bank-A chunks then bank-B chunks); un-permute to leaf order.
    SLOT = [0, 2, 1, 3, 4, 6, 5, 7]

    def gather(o):
        outs = []
        for r in range(N_CORES):
            block = res.results[r]["out"][:, 8 * o : 8 * o + 8]  # [128, 8]
            outs.append(block[:, SLOT].T.reshape(-1))            # leaf-major
        return np.concatenate(outs)

    d1min = gather(0)
    d2min = gather(1)

    def finalize(dmin2, sorted_q, other, Bg):
        d = np.sqrt(np.maximum(dmin2, 0.0).astype(np.float64))
        bound = np.repeat(Bg, 128)
        bad = np.nonzero(d * (1.0 + 1e-3) + 1e-6 > bound)[0]
        if len(bad):
            diff = sorted_q[bad, None, :] - other[None, :, :]
            d[bad] = np.sqrt((diff * diff).sum(-1).min(axis=1))
        return d

    d1 = finalize(d1min, s1s, s2_64, B1)
    d2 = finalize(d2min, s2s, s1_64, B2)
    return d1, d2, res


def kernel(set1, set2, hausdorff=0, w_set1_set2=1, w_set2_set1=1, n_outputs=1):
    s1 = np.ascontiguousarray(np.asarray(set1, dtype=np.float32))
    s2 = np.ascontiguousarray(np.asarray(set2, dtype=np.float32))
    assert s1.shape == (N, D) and s2.shape == (M, D), (s1.shape, s2.shape)
    hausdorff = int(np.asarray(hausdorff))
    w12 = int(np.asarray(w_set1_set2))
    w21 = int(np.asarray(w_set2_set1))
    n_outputs = int(np.asarray(n_outputs))

    d1, d2, _ = _run_device(s1, s2)

    reduce = np.mean if hausdorff == 0 else np.max
    t12 = np.float32(reduce(d1)) if w12 != 0 else np.float32(0.0)
    t21 = np.float32(reduce(d2)) if w21 != 0 else np.float32(0.0)

    if n_outputs == 1:
        return np.float32(t12 + t21)
    return (t12, t21)
